# revision 8
# baseline (speedup 1.0000x reference)
"""Locally-connected 1D conv (per-output-position weights) on 8 trn2 NeuronCores.

out[b,d,o] = relu(sum_{c,k} x[b,c,o+k] * w[d,c,o,k] + bias[d])
B=16, C=32, D=32, K=16, O=8176 (IN=8192).

Strategy: shard the output dimension O across 8 cores (1022 each). w (535MB)
dominates HBM traffic and every element is used exactly once, so the kernel is
DMA-bound; the job is to minimize resident bytes (tolerance 2e-2 rms):
  - w chunks for k=0..7 are bf16, k=8..15 are fp8e4m3 (measured output rms
    error 1.63e-2 on the fixed inputs, vs 2.0e-3 for all-bf16).
  - x is loaded RAW once as bf16 ([32 x XWIN*B]); the 4x-shifted im2col the
    matmuls need is built on-device by the otherwise-idle VectorE (3
    partition-group shift copies), saving 3x of x's DMA bytes.
  - outputs ship back as bf16.
Per output position o: 4 accumulating matmuls with contraction
(khat4, c32)=128; w-chunk [128x32] stationary (bf16 for q=0,1, fp8 for
q=2,3), the x-window [128x16] bf16 moving; PSUM holds [d32 x b16] per o,
32 o's per bank. ScalarE evacuates with fused bias+ReLU straight to bf16
(VectorE handles the final block so the two tail chains overlap), and the
final out-DMA goes on the idle sync queue.
"""

import numpy as np
import ml_dtypes

import concourse.bacc as bacc
import concourse.mybir as mybir
from concourse import bass_utils
from concourse.bass import ds
from concourse.tile import TileContext

B, C, D, K, O, IN = 16, 32, 32, 16, 8176, 8192
NCORES = 8
OSH = O // NCORES  # 1022 outputs per core
SLEN = OSH + (K - 4)  # 1034 window-start positions (s = o + 4q, q<4)
XWIN = OSH + K - 1  # 1037 x columns needed per core
PT = 32  # outputs per PSUM tile (32*16=512 f32 = one bank)
OT = 64  # outputs per w DMA block

BF16 = ml_dtypes.bfloat16
FP8 = ml_dtypes.float8_e4m3fn

_CACHE = {}


def _block_sizes():
    # small first block so the PE starts early; shrinking tail blocks so the
    # post-last-DMA matmul->evac->out chain is short
    sizes = [16] + [OT] * 15
    rem = OSH - sum(sizes)  # 46
    sizes += [rem - 16, 16]
    assert sum(sizes) == OSH and min(sizes) > 0
    return sizes


def _build():
    if "nc" in _CACHE:
        return _CACHE["nc"]
    nc = bacc.Bacc("TRN2", target_bir_lowering=False, debug=False)
    f32 = mybir.dt.float32
    bf = mybir.dt.bfloat16
    f8 = mybir.dt.float8e4
    # w chunks q=0,1 (k=0..7) bf16; q=2,3 (k=8..15) fp8e4m3
    w2b = nc.dram_tensor("w2b", (128, OSH * 2 * 32), bf, kind="ExternalInput")
    w2f = nc.dram_tensor("w2f", (128, OSH * 2 * 32), f8, kind="ExternalInput")
    xr = nc.dram_tensor("xr", (32, XWIN * B), bf, kind="ExternalInput")
    bias = nc.dram_tensor("bias", (D, 1), f32, kind="ExternalInput")
    out = nc.dram_tensor("out", (D, OSH * B), bf, kind="ExternalOutput")

    sizes = _block_sizes()
    offs = [sum(sizes[:i]) for i in range(len(sizes))]
    nblk = len(sizes)

    with TileContext(nc) as tc:
        with (
            tc.tile_pool(name="const", bufs=1) as cpool,
            tc.tile_pool(name="wpool", bufs=4) as wpool,
            tc.tile_pool(name="opool", bufs=3) as opool,
            tc.tile_pool(name="psum", bufs=8, space="PSUM") as ppool,
        ):
            # first w block DMA issued first so DMA_ENGINES starts ASAP
            wts = {}
            for bi in (0,):
                wtb = wpool.tile([128, OT * 64], bf, tag="wb")
                wtf = wpool.tile([128, OT * 64], f8, tag="wf")
                nc.sync.dma_start(
                    out=wtb[:, : sizes[bi] * 64],
                    in_=w2b[:, ds(offs[bi] * 64, sizes[bi] * 64)],
                )
                nc.sync.dma_start(
                    out=wtf[:, : sizes[bi] * 64],
                    in_=w2f[:, ds(offs[bi] * 64, sizes[bi] * 64)],
                )
                wts[bi] = (wtb, wtf)

            s_tile = cpool.tile([128, XWIN * B], bf)
            # raw x into partition group 0 (khat=0), in 4 chunks so the
            # VectorE shift-copies (and then the first matmuls) start early
            NCH = 4
            cs = XWIN * B // NCH  # 4148
            for c0 in range(0, XWIN * B, cs):
                nc.scalar.dma_start(
                    out=s_tile[0:32, ds(c0, cs)], in_=xr[:, ds(c0, cs)]
                )
            # build khat=1..3 groups as shifted copies of group 0 (keeps 3/4
            # of the im2col off the DMA bus)
            ccs = SLEN * B // NCH  # 4136
            for j in range(NCH):
                j0 = j * ccs
                cn = ccs if j < NCH - 1 else SLEN * B - j0
                for kh in range(1, 4):
                    nc.vector.tensor_copy(
                        out=s_tile[ds(32 * kh, 32), ds(j0, cn)],
                        in_=s_tile[0:32, ds(j0 + kh * B, cn)],
                    )
            b_tile = cpool.tile([D, 1], f32)
            nc.scalar.dma_start(out=b_tile[:, :], in_=bias[:, :])

            for bi, (o0, no) in enumerate(zip(offs, sizes)):
                if bi in wts:
                    wtb, wtf = wts[bi]
                else:
                    wtb = wpool.tile([128, OT * 64], bf, tag="wb")
                    wtf = wpool.tile([128, OT * 64], f8, tag="wf")
                    nc.sync.dma_start(
                        out=wtb[:, : no * 64], in_=w2b[:, ds(o0 * 64, no * 64)]
                    )
                    nc.sync.dma_start(
                        out=wtf[:, : no * 64], in_=w2f[:, ds(o0 * 64, no * 64)]
                    )
                last = bi % 2 == 1  # alternate evac/out chains across engines
                ot = opool.tile([D, OT * B], mybir.dt.bfloat16, tag="ot")
                for p0 in range(0, no, PT):
                    np_ = min(PT, no - p0)
                    psum = ppool.tile([D, PT * B], mybir.dt.float32, tag="ps")
                    for ol in range(p0, p0 + np_):
                        o = o0 + ol
                        for q in range(4):
                            wt = wtb if q < 2 else wtf
                            nc.tensor.matmul(
                                psum[:, ds((ol - p0) * B, B)],
                                wt[:, ds(ol * 64 + (q % 2) * 32, 32)],
                                s_tile[:, ds((o + 4 * q) * B, B)],
                                start=(q == 0),
                                stop=(q == 3),
                            )
                    if last:
                        # VectorE evac + sync-queue out-DMA: overlaps with the
                        # ScalarE chain of the previous block
                        nc.vector.tensor_scalar(
                            ot[:, ds(p0 * B, np_ * B)],
                            psum[:, : np_ * B],
                            b_tile[:, :],
                            0.0,
                            mybir.AluOpType.add,
                            mybir.AluOpType.max,
                        )
                    else:
                        nc.scalar.activation(
                            ot[:, ds(p0 * B, np_ * B)],
                            psum[:, : np_ * B],
                            mybir.ActivationFunctionType.Relu,
                            bias=b_tile[:, :],
                            scale=1.0,
                        )
                eng = nc.gpsimd
                eng.dma_start(
                    out=out[:, ds(o0 * B, no * B)], in_=ot[:, : no * B]
                )

    nc.compile()
    _CACHE["nc"] = nc
    return nc


def _pack_core(x, w, b, i):
    o0 = i * OSH
    # w2[p=(khat*32+c)][o][q][d] = w[d, c, o0+o, 4q+khat]; q=0,1 -> bf16
    # tensor, q=2,3 -> fp8 tensor
    wi = w[:, :, o0 : o0 + OSH, :]  # (D, C, OSH, K)
    a = wi.transpose(3, 1, 2, 0)  # (K, C, OSH, D) = [k][c][o][d]
    a = a.reshape(4, 4, C, OSH, D)  # [q][khat][c][o][d]
    a = a.transpose(1, 2, 3, 0, 4)  # [khat][c][o][q][d]
    w2b = np.ascontiguousarray(a[:, :, :, :2, :].reshape(128, OSH * 2 * D)).astype(BF16)
    w2f = np.ascontiguousarray(a[:, :, :, 2:, :].reshape(128, OSH * 2 * D)).astype(FP8)
    # xr[c][s*B+b] = x[b, c, o0+s]
    xs = x[:, :, o0 : o0 + XWIN]  # (B, C, XWIN)
    xr = np.ascontiguousarray(xs.transpose(1, 2, 0).reshape(C, XWIN * B)).astype(BF16)
    bias = np.ascontiguousarray(b.reshape(D, 1), dtype=np.float32)
    return {"w2b": w2b, "w2f": w2f, "xr": xr, "bias": bias}


def kernel(x, w, b, _results_hook=None):
    x = np.asarray(x, dtype=np.float32)
    w = np.asarray(w, dtype=np.float32)
    b = np.asarray(b, dtype=np.float32)
    nc = _build()
    in_maps = [_pack_core(x, w, b, i) for i in range(NCORES)]
    import os

    trace = bool(int(os.environ.get("KTRACE", "0")))
    res = bass_utils.run_bass_kernel_spmd(
        nc, in_maps, core_ids=list(range(NCORES)), trace=trace
    )
    if _results_hook is not None:
        _results_hook(res)
    parts = []
    for i in range(NCORES):
        oi = res.results[i]["out"].astype(np.float32).reshape(D, OSH, B)
        parts.append(oi.transpose(2, 0, 1))  # (B, D, OSH)
    return np.ascontiguousarray(np.concatenate(parts, axis=2))


# revision 9
# speedup vs baseline: 1.0602x; 1.0602x over previous
"""Locally-connected 1D conv (per-output-position weights) on 8 trn2 NeuronCores.

out[b,d,o] = relu(sum_{c,k} x[b,c,o+k] * w[d,c,o,k] + bias[d])
B=16, C=32, D=32, K=16, O=8176 (IN=8192).

Strategy: shard the output dimension O across 8 cores (1022 each). w (535MB)
dominates HBM traffic and every element is used exactly once, so the kernel is
DMA-bound; the job is to minimize resident bytes (tolerance 2e-2 rms):
  - w chunks for k=0..7 are bf16, k=8..15 are fp8e4m3 (measured output rms
    error 1.63e-2 on the fixed inputs, vs 2.0e-3 for all-bf16).
  - x is loaded RAW once as bf16 ([32 x XWIN*B]); the 4x-shifted im2col the
    matmuls need is built on-device by the otherwise-idle VectorE (3
    partition-group shift copies), saving 3x of x's DMA bytes.
  - outputs ship back as bf16.
Per output position o: 4 accumulating matmuls with contraction
(khat4, c32)=128; w-chunk [128x32] stationary (bf16 for q=0,1, fp8 for
q=2,3), the x-window [128x16] bf16 moving; PSUM holds [d32 x b16] per o,
32 o's per bank. ScalarE evacuates with fused bias+ReLU straight to bf16
(VectorE handles the final block so the two tail chains overlap), and the
final out-DMA goes on the idle sync queue.
"""

import numpy as np
import ml_dtypes

import concourse.bacc as bacc
import concourse.mybir as mybir
from concourse import bass_utils
from concourse.bass import ds
from concourse.tile import TileContext

B, C, D, K, O, IN = 16, 32, 32, 16, 8176, 8192
NCORES = 8
OSH = O // NCORES  # 1022 outputs per core
SLEN = OSH + (K - 4)  # 1034 window-start positions (s = o + 4q, q<4)
XWIN = OSH + K - 1  # 1037 x columns needed per core
PT = 32  # outputs per PSUM tile (32*16=512 f32 = one bank)
OT = 64  # outputs per w DMA block

BF16 = ml_dtypes.bfloat16
FP8 = ml_dtypes.float8_e4m3fn

_CACHE = {}


def _block_sizes():
    # small first block so the PE starts early; shrinking tail blocks so the
    # post-last-DMA matmul->evac->out chain is short
    sizes = [16] + [OT] * 15
    rem = OSH - sum(sizes)  # 46
    sizes += [rem - 16, 16]
    assert sum(sizes) == OSH and min(sizes) > 0
    return sizes


def _build():
    if "nc" in _CACHE:
        return _CACHE["nc"]
    nc = bacc.Bacc("TRN2", target_bir_lowering=False, debug=False)
    f32 = mybir.dt.float32
    bf = mybir.dt.bfloat16
    f8 = mybir.dt.float8e4
    # w chunks q=0,1 (k=0..7) bf16; q=2,3 (k=8..15) fp8e4m3
    w2b = nc.dram_tensor("w2b", (128, OSH * 2 * 32), bf, kind="ExternalInput")
    w2f = nc.dram_tensor("w2f", (128, OSH * 2 * 32), f8, kind="ExternalInput")
    xr = nc.dram_tensor("xr", (32, XWIN * B), bf, kind="ExternalInput")
    bias = nc.dram_tensor("bias", (D, 1), f32, kind="ExternalInput")
    out = nc.dram_tensor("out", (D, OSH * B), bf, kind="ExternalOutput")

    sizes = _block_sizes()
    offs = [sum(sizes[:i]) for i in range(len(sizes))]
    nblk = len(sizes)

    with TileContext(nc) as tc:
        with (
            tc.tile_pool(name="const", bufs=1) as cpool,
            tc.tile_pool(name="wpool", bufs=4) as wpool,
            tc.tile_pool(name="opool", bufs=6) as opool,
            tc.tile_pool(name="psum", bufs=8, space="PSUM") as ppool,
        ):
            # first w block DMA issued first so DMA_ENGINES starts ASAP
            wts = {}
            for bi in (0,):
                wtb = wpool.tile([128, OT * 64], bf, tag="wb")
                wtf = wpool.tile([128, OT * 64], f8, tag="wf")
                nc.sync.dma_start(
                    out=wtb[:, : sizes[bi] * 64],
                    in_=w2b[:, ds(offs[bi] * 64, sizes[bi] * 64)],
                )
                nc.sync.dma_start(
                    out=wtf[:, : sizes[bi] * 64],
                    in_=w2f[:, ds(offs[bi] * 64, sizes[bi] * 64)],
                )
                wts[bi] = (wtb, wtf)

            s_tile = cpool.tile([128, XWIN * B], bf)
            # raw x into partition group 0 (khat=0), in 4 chunks so the
            # VectorE shift-copies (and then the first matmuls) start early
            NCH = 4
            cs = XWIN * B // NCH  # 4148
            for c0 in range(0, XWIN * B, cs):
                nc.scalar.dma_start(
                    out=s_tile[0:32, ds(c0, cs)], in_=xr[:, ds(c0, cs)]
                )
            # build khat=1..3 groups as shifted copies of group 0 (keeps 3/4
            # of the im2col off the DMA bus)
            ccs = SLEN * B // NCH  # 4136
            for j in range(NCH):
                j0 = j * ccs
                cn = ccs if j < NCH - 1 else SLEN * B - j0
                for kh in range(1, 4):
                    nc.vector.tensor_copy(
                        out=s_tile[ds(32 * kh, 32), ds(j0, cn)],
                        in_=s_tile[0:32, ds(j0 + kh * B, cn)],
                    )
            b_tile = cpool.tile([D, 1], f32)
            nc.scalar.dma_start(out=b_tile[:, :], in_=bias[:, :])

            for bi, (o0, no) in enumerate(zip(offs, sizes)):
                if bi in wts:
                    wtb, wtf = wts[bi]
                else:
                    wtb = wpool.tile([128, OT * 64], bf, tag="wb")
                    wtf = wpool.tile([128, OT * 64], f8, tag="wf")
                    nc.sync.dma_start(
                        out=wtb[:, : no * 64], in_=w2b[:, ds(o0 * 64, no * 64)]
                    )
                    nc.sync.dma_start(
                        out=wtf[:, : no * 64], in_=w2f[:, ds(o0 * 64, no * 64)]
                    )
                last = bi % 2 == 1  # alternate evac/out chains across engines
                if bi % 2 == 0:
                    ot = opool.tile([D, 2 * OT * B], mybir.dt.bfloat16, tag="ot")
                    ot_off, ot_o0 = 0, o0
                else:
                    ot_off = (o0 - ot_o0) * B
                for p0 in range(0, no, PT):
                    np_ = min(PT, no - p0)
                    psum = ppool.tile([D, PT * B], mybir.dt.float32, tag="ps")
                    for ol in range(p0, p0 + np_):
                        o = o0 + ol
                        for q in range(4):
                            wt = wtb if q < 2 else wtf
                            nc.tensor.matmul(
                                psum[:, ds((ol - p0) * B, B)],
                                wt[:, ds(ol * 64 + (q % 2) * 32, 32)],
                                s_tile[:, ds((o + 4 * q) * B, B)],
                                start=(q == 0),
                                stop=(q == 3),
                            )
                    if last:
                        # VectorE evac overlaps with the ScalarE chain of the
                        # neighboring block
                        nc.vector.tensor_scalar(
                            ot[:, ds(ot_off + p0 * B, np_ * B)],
                            psum[:, : np_ * B],
                            b_tile[:, :],
                            0.0,
                            mybir.AluOpType.add,
                            mybir.AluOpType.max,
                        )
                    else:
                        nc.scalar.activation(
                            ot[:, ds(ot_off + p0 * B, np_ * B)],
                            psum[:, : np_ * B],
                            mybir.ActivationFunctionType.Relu,
                            bias=b_tile[:, :],
                            scale=1.0,
                        )
                if bi % 2 == 1 or bi == nblk - 1:
                    # one out-DMA per block pair, alternating HWDGE queues
                    tot = o0 + no - ot_o0
                    eng = nc.gpsimd if (bi // 2) % 2 else nc.scalar
                    eng.dma_start(
                        out=out[:, ds(ot_o0 * B, tot * B)], in_=ot[:, : tot * B]
                    )

    nc.compile()
    _CACHE["nc"] = nc
    return nc


def _pack_core(x, w, b, i):
    o0 = i * OSH
    # w2[p=(khat*32+c)][o][q][d] = w[d, c, o0+o, 4q+khat]; q=0,1 -> bf16
    # tensor, q=2,3 -> fp8 tensor
    wi = w[:, :, o0 : o0 + OSH, :]  # (D, C, OSH, K)
    a = wi.transpose(3, 1, 2, 0)  # (K, C, OSH, D) = [k][c][o][d]
    a = a.reshape(4, 4, C, OSH, D)  # [q][khat][c][o][d]
    a = a.transpose(1, 2, 3, 0, 4)  # [khat][c][o][q][d]
    w2b = np.ascontiguousarray(a[:, :, :, :2, :].reshape(128, OSH * 2 * D)).astype(BF16)
    w2f = np.ascontiguousarray(a[:, :, :, 2:, :].reshape(128, OSH * 2 * D)).astype(FP8)
    # xr[c][s*B+b] = x[b, c, o0+s]
    xs = x[:, :, o0 : o0 + XWIN]  # (B, C, XWIN)
    xr = np.ascontiguousarray(xs.transpose(1, 2, 0).reshape(C, XWIN * B)).astype(BF16)
    bias = np.ascontiguousarray(b.reshape(D, 1), dtype=np.float32)
    return {"w2b": w2b, "w2f": w2f, "xr": xr, "bias": bias}


def kernel(x, w, b, _results_hook=None):
    x = np.asarray(x, dtype=np.float32)
    w = np.asarray(w, dtype=np.float32)
    b = np.asarray(b, dtype=np.float32)
    nc = _build()
    in_maps = [_pack_core(x, w, b, i) for i in range(NCORES)]
    import os

    trace = bool(int(os.environ.get("KTRACE", "0")))
    res = bass_utils.run_bass_kernel_spmd(
        nc, in_maps, core_ids=list(range(NCORES)), trace=trace
    )
    if _results_hook is not None:
        _results_hook(res)
    parts = []
    for i in range(NCORES):
        oi = res.results[i]["out"].astype(np.float32).reshape(D, OSH, B)
        parts.append(oi.transpose(2, 0, 1))  # (B, D, OSH)
    return np.ascontiguousarray(np.concatenate(parts, axis=2))
